# revision 33
# baseline (speedup 1.0000x reference)
"""Trainium2 Bass kernel for nn_DocumentHead (retrieval head MLP).

Math (per batch row):
    align = <v_claim, v_doc> / (max(||v_claim||,eps) * max(||v_doc||,eps))
    div   = 1 - align ; tens = div^2
    h      = relu([h_final | align | div | tens] @ W1 + b1)
    shared = relu(h @ W2 + b2)
    out    = sigmoid(shared @ Wr + br)

Strategy: data-parallel over batch on 8 cores (2048 rows/core). All dtype
conversion and data transposition happens on the HOST inside kernel() —
the device program only streams pre-laid-out operands and runs matmuls.

Numerics: stages 1 and 2 run as fp8e4m3 DoubleRow matmuls (0.5 cyc/row,
2 k-tiles per instruction = 4x bf16 PE throughput) with a 3-term
error-split per stage:  x@W ~= x1@Whi + xr@Whi + x1@Wlo  where x1/Whi are
fp8 quantizations (at fixed power-of-2 scales) and xr/Wlo the fp8-encoded
residuals AT THE SAME SCALE, so all three streams accumulate into one
PSUM group. This recovers bf16-level accuracy (measured 2.2e-3 max rel
err vs 2.4e-3 for all-bf16) at ~1/3 the bf16 PE cost. The [align,div,
tens] extras enter stage 1 as one extra DoubleRow pair-tile (3 live
contraction rows zero-padded). h is written once by ACT as bf16
(relu(psum/64 + 4*b1) = 4*relu(x@W1+b1)); DVE then derives the fp8
stage-2 operand pair h1 = fp8(h), hr = fp8(h - h1). Stage 3 (K=1024 dot
with Wr) stays bf16. Cosine stats accumulate in f32 from the fp8 v
tensors (the quant error there is ~5e-4 on align and contributes ~1e-5
downstream through the tiny W1 extra rows).

Host prep per core: h_final*8 -> fp8 hi+residual, both pre-TRANSPOSED to
[D, bc] so k-tiles DMA straight into the matmul layout; v_claim/v_doc*4
-> fp8; W1*32/W2*32 -> fp8 hi+lo in k-major layout; biases pre-shaped to
[128, n] tiles (b1 pre-scaled by 4 to fold the x-scale).

DMA per core: 8.4 MB x-splits + 8.4 MB v + 12.6 MB weights = 29.4 MB
(vs 75.6 MB for the f32 baseline). The prologue interleaves x1/W1hi
k-pair chunks so the first stage-1 stream starts after ~0.8 MB of DMA.
Engine load per superchunk: PE ~65 us (the bottleneck), ACT ~20 us,
DVE ~25 us; gpsimd (Q7, 2.8 us sequencer launch per op) is kept out of
the hot path entirely.
"""

import numpy as np

P = 128
D = 2048
NCORES = 8
FREE = 512          # moving free dim / batch-chunk width
KT = D // P         # 16 k-tiles for stage 1 contraction
PAIRS = KT // 2     # 8 DoubleRow k-pairs
NT = D // P         # 16 n-tiles  (stage-1 output features)
J = D // 2          # 1024
JT = J // P         # 8 j-tiles  (stage-2 output features)
HD = D // 2         # stats half width
EPS = 1e-12

XS = 8.0            # h_final fp8 scale
VS = 4.0            # v_claim/v_doc fp8 scale
WS = 32.0           # W1/W2 fp8 scale
HS = 4.0            # h (stage-2 input) fp8 scale

_cache = {}


def _build(bc, reps=1):
    """Build the per-core Bass program for bc batch rows.

    reps > 1 repeats the whole pipeline over the same inputs inside one
    NEFF — used only for timing (amortizes host dispatch overhead).
    """
    import concourse.bass as bass
    import concourse.tile as tile
    from concourse import bacc, mybir
    from concourse.masks import make_identity

    f32 = mybir.dt.float32
    bf16 = mybir.dt.bfloat16
    f8 = mybir.dt.float8e4
    AF = mybir.ActivationFunctionType
    OP = mybir.AluOpType
    DR = mybir.MatmulPerfMode.DoubleRow

    nsc = bc // FREE            # super-chunks (= batch chunks) per core
    nmt = FREE // P             # m-tiles per super-chunk (4)

    nc = bacc.Bacc(trn_type="TRN2", target_bir_lowering=False, debug=False)

    x1t = nc.dram_tensor("x1t", [D, bc], f8, kind="ExternalInput").ap()
    xrt = nc.dram_tensor("xrt", [D, bc], f8, kind="ExternalInput").ap()
    vc8 = nc.dram_tensor("vc8", [bc, D], f8, kind="ExternalInput").ap()
    vd8 = nc.dram_tensor("vd8", [bc, D], f8, kind="ExternalInput").ap()
    w1hi = nc.dram_tensor("w1hi", [D, D], f8, kind="ExternalInput").ap()
    w1lo = nc.dram_tensor("w1lo", [D, D], f8, kind="ExternalInput").ap()
    w2hi = nc.dram_tensor("w2hi", [D, J], f8, kind="ExternalInput").ap()
    w2lo = nc.dram_tensor("w2lo", [D, J], f8, kind="ExternalInput").ap()
    exw = nc.dram_tensor("exw", [P, 2 * D], f8, kind="ExternalInput").ap()
    b1s = nc.dram_tensor("b1s", [P, NT], f32, kind="ExternalInput").ap()
    b2s = nc.dram_tensor("b2s", [P, JT], f32, kind="ExternalInput").ap()
    wrs = nc.dram_tensor("wrs", [P, JT], bf16, kind="ExternalInput").ap()
    brs = nc.dram_tensor("brs", [1, 1], f32, kind="ExternalInput").ap()
    out = nc.dram_tensor("out", [bc, 1], f32, kind="ExternalOutput").ap()

    # k-major DRAM views that DMA straight into [P, kt, free] SBUF tiles
    x1v = x1t.rearrange("(kt p) m -> p kt m", p=P)
    xrv = xrt.rearrange("(kt p) m -> p kt m", p=P)
    w1hiv = w1hi.rearrange("(kt p) d -> p kt d", p=P)
    w1lov = w1lo.rearrange("(kt p) d -> p kt d", p=P)
    w2hiv = w2hi.rearrange("(kt p) j -> p kt j", p=P)
    w2lov = w2lo.rearrange("(kt p) j -> p kt j", p=P)

    with tile.TileContext(nc) as tc:
        with (
            tc.tile_pool(name="singles", bufs=1) as singles,
            tc.tile_pool(name="xt", bufs=2) as xt_pool,
            tc.tile_pool(name="vt", bufs=2) as vt_pool,
            tc.tile_pool(name="ht", bufs=2) as ht_pool,
            tc.tile_pool(name="hbfp", bufs=1) as hbf_pool,
            tc.tile_pool(name="st", bufs=1) as st_pool,
            tc.tile_pool(name="stats", bufs=2) as stats,
            tc.tile_pool(name="psA", bufs=1, space="PSUM") as psA,
            tc.tile_pool(name="psB", bufs=2, space="PSUM") as psB,
        ):
            identb = singles.tile([P, P], bf16)
            make_identity(nc, identb)
            # stationary operands, DMA'd directly in final layout
            w1hi_sb = singles.tile([P, KT, D], f8)
            w1lo_sb = singles.tile([P, KT, D], f8)
            w2hi_sb = singles.tile([P, KT, J], f8)
            w2lo_sb = singles.tile([P, KT, J], f8)
            ex_sb = singles.tile([P, 2, D], f8)
            b1sb = singles.tile([P, NT], f32)
            b2sb = singles.tile([P, JT], f32)
            wrsb = singles.tile([P, JT], bf16)
            brsb = singles.tile([1, 1], f32)
            # [3 feat rows | zeros] x [pair, batch] — rhs of the extras
            # DoubleRow matmul. Zero everywhere a weight row is zero so
            # 0*garbage can't produce NaN in PSUM.
            featsT = singles.tile([P, 2, bc], f8)
            nc.vector.memset(featsT, 0.0)

            sc_state = {}

            def rowbase(sc):
                return (sc % nsc) * nmt

            def phaseA_start(sc, defer_x=False):
                s = sc_state[sc] = dict(
                    ccs=stats.tile([P, nmt], f32, tag="ccs", name=f"ccs{sc}"),
                    dds=stats.tile([P, nmt], f32, tag="dds", name=f"dds{sc}"),
                    cds=stats.tile([P, nmt], f32, tag="cds", name=f"cds{sc}"),
                    x1=xt_pool.tile([P, KT, FREE], f8, tag="x1", name=f"x1_{sc}"),
                    xr=xt_pool.tile([P, KT, FREE], f8, tag="xr", name=f"xr_{sc}"),
                )
                if not defer_x:
                    cols = slice((sc % nsc) * FREE, (sc % nsc + 1) * FREE)
                    nc.sync.dma_start(s["x1"], x1v[:, :, cols])
                    nc.sync.dma_start(s["xr"], xrv[:, :, cols])

            def phaseA_v(sc, mt):
                # cosine stats for one m-tile, in f32, split into halves.
                # DVE: cd mult+reduce and dd mult+reduce; ACT: cc Square
                # with accumulator — keeps any single op under ~1 us so the
                # in-order ACT queue can't stall PSUM-bank releases long.
                s = sc_state[sc]
                row = (rowbase(sc) + mt) * P
                hsum = stats.tile([P, 3, 2], f32, tag="hsum", name=f"hs{sc}_{mt}")
                for hh in range(2):
                    cols = slice(hh * HD, (hh + 1) * HD)
                    vcf = vt_pool.tile([P, HD], f8, tag="vcf",
                                       name=f"vc{sc}_{mt}{hh}")
                    nc.sync.dma_start(vcf, vc8[row:row + P, cols])
                    vdf = vt_pool.tile([P, HD], f8, tag="vdf",
                                       name=f"vd{sc}_{mt}{hh}")
                    nc.sync.dma_start(vdf, vd8[row:row + P, cols])
                    tr = vt_pool.tile([P, HD], bf16, tag="tr",
                                      name=f"tr{sc}_{mt}{hh}")
                    nc.vector.tensor_mul(tr, vcf, vdf)
                    nc.vector.reduce_sum(hsum[:, 0, hh:hh + 1], tr,
                                         axis=mybir.AxisListType.X)
                    tr2 = vt_pool.tile([P, HD], bf16, tag="tr2",
                                       name=f"t2{sc}_{mt}{hh}")
                    nc.scalar.activation(tr2, vcf, AF.Square,
                                         accum_out=hsum[:, 1, hh:hh + 1])
                    nc.scalar.activation(tr2, vdf, AF.Square,
                                         accum_out=hsum[:, 2, hh:hh + 1])
                nc.vector.reduce_sum(s["cds"][:, mt:mt + 1], hsum[:, 0, :],
                                     axis=mybir.AxisListType.X)
                nc.vector.reduce_sum(s["ccs"][:, mt:mt + 1], hsum[:, 1, :],
                                     axis=mybir.AxisListType.X)
                nc.vector.reduce_sum(s["dds"][:, mt:mt + 1], hsum[:, 2, :],
                                     axis=mybir.AxisListType.X)

            def phaseA_finish(sc):
                # stats -> fp8 [align, div, tens]*XS rows of featsT
                s = sc_state[sc]
                ccs, dds, cds = s["ccs"], s["dds"], s["cds"]
                feats = stats.tile([P, nmt, 3], f32, tag="feats", name=f"ft{sc}")
                featsb = stats.tile([P, nmt, 3], bf16, tag="featsb",
                                    name=f"fb{sc}")
                nc.scalar.activation(ccs, ccs, AF.Sqrt)
                nc.scalar.activation(dds, dds, AF.Sqrt)
                nc.vector.tensor_scalar_max(ccs, ccs, EPS)
                nc.vector.tensor_scalar_max(dds, dds, EPS)
                nc.vector.tensor_mul(ccs, ccs, dds)
                nc.vector.reciprocal(ccs, ccs)
                nc.vector.tensor_mul(feats[:, :, 0], cds, ccs)      # align
                nc.vector.tensor_scalar(feats[:, :, 1], feats[:, :, 0],
                                        -1.0, 1.0, OP.mult, OP.add)  # div
                nc.vector.tensor_mul(feats[:, :, 2], feats[:, :, 1],
                                     feats[:, :, 1])                 # tens
                nc.vector.tensor_scalar(featsb, feats, XS, None, OP.mult)
                for mt in range(nmt):
                    psf = psB.tile([3, P], bf16, tag="ps2", name=f"psf{sc}_{mt}")
                    nc.tensor.transpose(psf, featsb[:, mt, :], identb)
                    col = (rowbase(sc) + mt) * P
                    nc.vector.tensor_copy(featsT[0:3, 0, col:col + P], psf)

            # ---- prologue: interleave sc0's x1 k-pairs with W1hi k-pairs
            # so stage 1's first stream starts after <1 MB of DMA; v/xr/W1lo
            # and the W2 tensors stream underneath sc0's compute ----
            phaseA_start(0, defer_x=True)
            s0 = sc_state[0]
            c0 = slice(0, FREE)
            for k in range(PAIRS):
                pr = slice(2 * k, 2 * k + 2)
                nc.sync.dma_start(s0["x1"][:, pr, :], x1v[:, pr, c0])
                nc.sync.dma_start(w1hi_sb[:, pr, :], w1hiv[:, pr, :])
                if k == PAIRS - 1:
                    nc.sync.dma_start(b1sb, b1s)
                    nc.sync.dma_start(b2sb, b2s)
                    nc.sync.dma_start(wrsb, wrs)
                    nc.sync.dma_start(brsb, brs)
                    nc.sync.dma_start(
                        ex_sb, exw.rearrange("p (two d) -> p two d", two=2))
            for k in range(PAIRS):
                pr = slice(2 * k, 2 * k + 2)
                nc.sync.dma_start(s0["xr"][:, pr, :], xrv[:, pr, c0])
                if k % 2 == 0:
                    phaseA_v(0, k // 2)
            for k in range(PAIRS):
                pr = slice(2 * k, 2 * k + 2)
                nc.sync.dma_start(w1lo_sb[:, pr, :], w1lov[:, pr, :])
            # W2 streams lazily from inside sc0's quarter loop so sc1's
            # x/v DMAs (which feed in-order ACT/PE work) queue ahead of it
            phaseA_finish(0)

            total_sc = nsc * reps
            for sc in range(total_sc):
                nxt = sc + 1 if sc + 1 < total_sc else None
                if nxt is not None:
                    phaseA_start(nxt)
                mcols = slice((sc % nsc) * FREE, (sc % nsc + 1) * FREE)
                s = sc_state[sc]
                x1, xr = s["x1"], s["xr"]

                # ---- stage 1: 3-term fp8 DoubleRow into one PSUM group ----
                h1 = ht_pool.tile([P, NT, FREE], f8, tag="h1", name=f"h1_{sc}")
                hr = ht_pool.tile([P, NT, FREE], f8, tag="hr", name=f"hr_{sc}")
                # sc0's stage 1 is paced by the W1 DMA stream: run it with
                # 6-wide accumulator waves (6 banks + 2 for psB = all 8) so
                # compute-ready chunks overlap the W1lo DMA tail instead of
                # serializing behind it quarter by quarter
                if sc == 0:
                    chunks = [(0, 6), (6, 12), (12, 16)]
                else:
                    chunks = [(0, 4), (4, 8), (8, 12), (12, 16)]
                for ci, (nt0, nt1) in enumerate(chunks):
                    nts = range(nt0, nt1)
                    pss = {nt: psA.tile([P, FREE], f32, tag=f"ps1_{nt - nt0}",
                                        name=f"ps1_{sc}_{ci}_{nt - nt0}")
                           for nt in nts}
                    hbf = hbf_pool.tile([P, len(nts), FREE], bf16, tag="hbf",
                                        name=f"hbf{sc}_{ci}")
                    if sc == 0:
                        # stream-major: x1@w1hi first so sc0's stage 1
                        # tracks the prologue DMA order (x1, w1hi, xr, w1lo)
                        for pk in range(PAIRS):
                            for nt in nts:
                                nc.tensor.matmul(
                                    pss[nt],
                                    w1hi_sb[:, 2 * pk:2 * pk + 2,
                                            nt * P:(nt + 1) * P],
                                    x1[:, 2 * pk:2 * pk + 2, :],
                                    start=(pk == 0), stop=False, perf_mode=DR)
                        for pk in range(PAIRS):
                            for nt in nts:
                                nc.tensor.matmul(
                                    pss[nt],
                                    w1hi_sb[:, 2 * pk:2 * pk + 2,
                                            nt * P:(nt + 1) * P],
                                    xr[:, 2 * pk:2 * pk + 2, :],
                                    start=False, stop=False, perf_mode=DR)
                        for pk in range(PAIRS):
                            for nt in nts:
                                nc.tensor.matmul(
                                    pss[nt],
                                    w1lo_sb[:, 2 * pk:2 * pk + 2,
                                            nt * P:(nt + 1) * P],
                                    x1[:, 2 * pk:2 * pk + 2, :],
                                    start=False, stop=False, perf_mode=DR)
                    else:
                        # steady state: x1/xr back-to-back against the SAME
                        # w1hi tile — DoubleRow disables fast-weight-load,
                        # so consecutive same-stationary matmuls cut real
                        # LDWEIGHTS pressure (cost-model-neutral)
                        for pk in range(PAIRS):
                            for nt in nts:
                                w = w1hi_sb[:, 2 * pk:2 * pk + 2,
                                            nt * P:(nt + 1) * P]
                                nc.tensor.matmul(
                                    pss[nt], w, x1[:, 2 * pk:2 * pk + 2, :],
                                    start=(pk == 0), stop=False, perf_mode=DR)
                                nc.tensor.matmul(
                                    pss[nt], w, xr[:, 2 * pk:2 * pk + 2, :],
                                    start=False, stop=False, perf_mode=DR)
                        for pk in range(PAIRS):
                            for nt in nts:
                                nc.tensor.matmul(
                                    pss[nt],
                                    w1lo_sb[:, 2 * pk:2 * pk + 2,
                                            nt * P:(nt + 1) * P],
                                    x1[:, 2 * pk:2 * pk + 2, :],
                                    start=False, stop=False, perf_mode=DR)
                    for nt in nts:
                        nc.tensor.matmul(pss[nt],
                                         ex_sb[:, :, nt * P:(nt + 1) * P],
                                         featsT[:, :, mcols],
                                         start=False, stop=True, perf_mode=DR)
                        # psum = 256*(x@W1+feats@W1x); h = 4*relu(x@W1+b1)
                        nc.scalar.activation(hbf[:, nt - nt0, :], pss[nt],
                                             AF.Relu,
                                             bias=b1sb[:, nt:nt + 1],
                                             scale=HS / (XS * WS))
                        nc.vector.tensor_copy(h1[:, nt, :], hbf[:, nt - nt0, :])
                        nc.vector.tensor_sub(hr[:, nt, :], hbf[:, nt - nt0, :],
                                             h1[:, nt, :])
                    if sc == 0:
                        # w2lo first: it feeds stage 2's FIRST stream
                        kt_lo, kt_hi = [(0, 6), (6, 11), (11, 16)][ci]
                        kt_mid = (kt_lo + kt_hi + 1) // 2
                        for w2pr in (slice(kt_lo, kt_mid),
                                     slice(kt_mid, kt_hi)):
                            nc.sync.dma_start(w2lo_sb[:, w2pr, :],
                                              w2lov[:, w2pr, :])
                            nc.sync.dma_start(w2hi_sb[:, w2pr, :],
                                              w2hiv[:, w2pr, :])

                # ---- stage 2: 3-term fp8 DoubleRow ----
                # pk 6-7 (h tiles nt12-15, freshest out of stage 1) are
                # touched LAST across all three streams so stage 2 never
                # races the tail of stage 1's ACT/DVE h-chain
                st = st_pool.tile([P, JT, FREE], bf16)
                for jt in range(JT):
                    ps = psB.tile([P, FREE], f32, tag="ps2")
                    first = True
                    for pks in (range(0, PAIRS - 2), range(PAIRS - 2, PAIRS)):
                        for pk in pks:
                            nc.tensor.matmul(
                                ps,
                                w2lo_sb[:, 2 * pk:2 * pk + 2,
                                        jt * P:(jt + 1) * P],
                                h1[:, 2 * pk:2 * pk + 2, :],
                                start=first, stop=False, perf_mode=DR)
                            first = False
                        for pk in pks:
                            # h1/hr back-to-back on the same w2hi tile
                            w = w2hi_sb[:, 2 * pk:2 * pk + 2,
                                        jt * P:(jt + 1) * P]
                            nc.tensor.matmul(
                                ps, w, h1[:, 2 * pk:2 * pk + 2, :],
                                start=False, stop=False, perf_mode=DR)
                            nc.tensor.matmul(
                                ps, w, hr[:, 2 * pk:2 * pk + 2, :],
                                start=False, stop=(pk == PAIRS - 1),
                                perf_mode=DR)
                    nc.scalar.activation(st[:, jt, :], ps, AF.Relu,
                                         bias=b2sb[:, jt:jt + 1],
                                         scale=1.0 / (HS * WS))
                    # next-sc stats interleave here (not between stage-1
                    # quarters): their ACT Squares would otherwise block
                    # the in-order ACT queue ahead of the PSUM-releasing
                    # h activations while next-sc v DMAs are in flight
                    if nxt is not None and jt % 2 == 0:
                        phaseA_v(nxt, jt // 2)

                # next-sc feats transposes go here so they sit AFTER stage
                # 2 in the in-order PE queue — stage 2's ~20 us of matmuls
                # cover the next-sc stats-chain latency
                if nxt is not None:
                    phaseA_finish(nxt)

                # ---- stage 3: out[m] = sigmoid(Wr.T @ sT + br), bf16 ----
                psd = psB.tile([1, FREE], f32, tag="ps2")
                for jt in range(JT):
                    nc.tensor.matmul(psd, wrsb[:, jt:jt + 1], st[:, jt, :],
                                     start=(jt == 0), stop=(jt == JT - 1))
                osb = stats.tile([1, FREE], f32, tag="osb", name=f"osb{sc}")
                nc.scalar.activation(osb, psd, AF.Sigmoid, bias=brsb[0:1, 0:1])
                nc.sync.dma_start(
                    out.rearrange("m one -> one m")[:, mcols], osb)

    nc.compile()
    return nc


def get_nc(bc, reps=1):
    if (bc, reps) not in _cache:
        _cache[(bc, reps)] = _build(bc, reps)
    return _cache[(bc, reps)]


def _shim_axon_hooks():
    """antenv.axon_hooks is absent in this container; shim it so a
    BASS_TRACE=1 environment can't crash run_bass_kernel_spmd."""
    import sys
    import types
    try:
        import antenv
    except ImportError:
        return
    if "antenv.axon_hooks" not in sys.modules:
        try:
            import antenv.axon_hooks  # noqa: F401
        except ImportError:
            m = types.ModuleType("antenv.axon_hooks")
            m.get_axon_ntff_profile_hook = lambda: None
            sys.modules["antenv.axon_hooks"] = m
            antenv.axon_hooks = m


def prep_inputs(inputs):
    """Host-side quantization/layout. Returns per-core input maps."""
    import ml_dtypes

    f8 = np.dtype(ml_dtypes.float8_e4m3)
    bf16 = np.dtype(ml_dtypes.bfloat16)
    B = inputs["h_final"].shape[0]
    bc = B // NCORES

    hf = np.asarray(inputs["h_final"], dtype=np.float32)
    x8 = hf * XS
    x1 = x8.astype(f8)
    xr = (x8 - x1.astype(np.float32)).astype(f8)
    vc = (np.asarray(inputs["v_claim"], dtype=np.float32) * VS).astype(f8)
    vd = (np.asarray(inputs["v_doc"], dtype=np.float32) * VS).astype(f8)

    W1 = np.asarray(inputs["W1"], dtype=np.float32) * WS
    w1hi = W1[:D].astype(f8)
    w1lo = (W1[:D] - w1hi.astype(np.float32)).astype(f8)
    W2 = np.asarray(inputs["W2"], dtype=np.float32) * WS
    w2hi = W2.astype(f8)
    w2lo = (W2 - w2hi.astype(np.float32)).astype(f8)
    exw = np.zeros((P, 2, D), dtype=f8)
    exw[0:3, 0, :] = W1[D:D + 3].astype(f8)
    exw = exw.reshape(P, 2 * D)

    b1 = np.asarray(inputs["b1"], dtype=np.float32)
    b2 = np.asarray(inputs["b2"], dtype=np.float32)
    b1s = np.ascontiguousarray((HS * b1).reshape(NT, P).T)
    b2s = np.ascontiguousarray(b2.reshape(JT, P).T)
    wrs = np.ascontiguousarray(
        np.asarray(inputs["Wr"], dtype=np.float32)[:, 0].reshape(JT, P).T
    ).astype(bf16)
    brs = np.asarray(inputs["br"], dtype=np.float32).reshape(1, 1)

    shared = dict(w1hi=w1hi, w1lo=w1lo, w2hi=w2hi, w2lo=w2lo, exw=exw,
                  b1s=b1s, b2s=b2s, wrs=wrs, brs=brs)
    in_maps = []
    for c in range(NCORES):
        rows = slice(c * bc, (c + 1) * bc)
        m = dict(shared)
        m["x1t"] = np.ascontiguousarray(x1[rows].T)
        m["xrt"] = np.ascontiguousarray(xr[rows].T)
        m["vc8"] = np.ascontiguousarray(vc[rows])
        m["vd8"] = np.ascontiguousarray(vd[rows])
        in_maps.append(m)
    return in_maps, bc


def kernel(**inputs):
    _shim_axon_hooks()
    from concourse.bass_utils import run_bass_kernel_spmd

    in_maps, bc = prep_inputs(inputs)
    nc = get_nc(bc)
    res = run_bass_kernel_spmd(nc, in_maps, core_ids=list(range(NCORES)))
    return np.concatenate([r["out"] for r in res.results], axis=0)
